# revision 2
# baseline (speedup 1.0000x reference)
"""Masked attention-weight kernel (dense_transformer) for 8 TRN2 NeuronCores.

Computes, for inputs query/key [32,1024,512] f32, masks [32,1024] i32:
    q = relu(query @ Wq + bq); k = relu(key @ Wk + bk)
    w = softmax((q @ k^T)/sqrt(512) + key_mask_additive) * query_mask
Output: [32, 1024, 1024] f32.

Strategy: pure data-parallel over batch (4 batches/core, no collectives).
Host pre-transposes query/key to [B_local, D, L] and casts to bf16 so every
device matmul is transpose-free; compute is bf16 with f32 PSUM accumulation.
Softmax skips max-subtraction (logits bounded ~+-12; exp safe in f32). The
key mask is applied additively (-1e6) to the post-relu k-projection, which
makes masked logits ~-2e8 so exp underflows to exactly 0, and the ACT exp's
fused accum_out produces the masked row sum for free.
"""

import sys

sys.path.insert(0, "/opt/trn_rl_repo")

import numpy as np
import ml_dtypes
from contextlib import ExitStack

import concourse.bass as bass
import concourse.tile as tile
from concourse import bacc, mybir
from concourse.bass_utils import run_bass_kernel_spmd

P = 128
B, LQ, LK, D = 32, 1024, 1024, 512
NCORES = 8
BL = B // NCORES          # batches per core
NDT = D // P              # contraction tiles for projections
NET = D // P              # output-feature tiles (= S contraction tiles)
NIB = LQ // P             # 128-row blocks of S per batch
NH = LK // 512            # 512-col halves
SCALE = float(1.0 / np.sqrt(D))
MASKC = -1.0e6

F32 = mybir.dt.float32
BF16 = mybir.dt.bfloat16
AF = mybir.ActivationFunctionType

_CACHE = {}


def _body(tc, qT, kT, Wq, Wk, bq, bk, maskb, qm, out):
    nc = tc.nc
    with ExitStack() as ctx:
        consts = ctx.enter_context(tc.tile_pool(name="consts", bufs=1))
        wpool = ctx.enter_context(tc.tile_pool(name="w", bufs=1))
        inpool = ctx.enter_context(tc.tile_pool(name="inp", bufs=2))
        actpool = ctx.enter_context(tc.tile_pool(name="act", bufs=2))
        mpool = ctx.enter_context(tc.tile_pool(name="mask", bufs=2))
        epool = ctx.enter_context(tc.tile_pool(name="exp", bufs=3))
        opool = ctx.enter_context(tc.tile_pool(name="pout", bufs=3))
        stpool = ctx.enter_context(tc.tile_pool(name="stat", bufs=6))
        ppsum = ctx.enter_context(tc.tile_pool(name="ppsum", bufs=3, space="PSUM"))
        spsum = ctx.enter_context(tc.tile_pool(name="spsum", bufs=2, space="PSUM"))

        bq_sb = consts.tile([P, NET], F32)
        nc.sync.dma_start(out=bq_sb[:], in_=bq[:])
        bk_sb = consts.tile([P, NET], F32)
        nc.sync.dma_start(out=bk_sb[:], in_=bk[:])

        wq_sb, wk_sb = [], []
        for dt_ in range(NDT):
            t = wpool.tile([P, D], BF16, tag=f"wq{dt_}")
            nc.sync.dma_start(out=t[:], in_=Wq[dt_ * P:(dt_ + 1) * P, :])
            wq_sb.append(t)
        for dt_ in range(NDT):
            t = wpool.tile([P, D], BF16, tag=f"wk{dt_}")
            nc.sync.dma_start(out=t[:], in_=Wk[dt_ * P:(dt_ + 1) * P, :])
            wk_sb.append(t)

        def load_inputs(b):
            xk, xq = [], []
            for dt_ in range(NDT):
                t = inpool.tile([P, LK], BF16, tag=f"xk{dt_}")
                nc.sync.dma_start(out=t[:], in_=kT[b, dt_ * P:(dt_ + 1) * P, :])
                xk.append(t)
            mask_sb = mpool.tile([P, LK], BF16, tag="maskb")
            nc.sync.dma_start(out=mask_sb[:], in_=maskb[b])
            for dt_ in range(NDT):
                t = inpool.tile([P, LQ], BF16, tag=f"xq{dt_}")
                nc.sync.dma_start(out=t[:], in_=qT[b, dt_ * P:(dt_ + 1) * P, :])
                xq.append(t)
            qm_sb = mpool.tile([P, NIB], F32, tag="qm")
            nc.sync.dma_start(out=qm_sb[:], in_=qm[b])
            return xk, mask_sb, xq, qm_sb

        def proj(xin, w_sb, bias_sb, out_tiles):
            # out_tiles[et][:, ih*512:+512] = relu(W[:,et].T @ x[:, ih] + b)
            for ih in range(NH):
                for et in range(NET):
                    ps = ppsum.tile([P, 512], F32, tag="proj")
                    for dt_ in range(NDT):
                        nc.tensor.matmul(
                            ps[:],
                            lhsT=w_sb[dt_][:, et * P:(et + 1) * P],
                            rhs=xin[dt_][:, ih * 512:(ih + 1) * 512],
                            start=(dt_ == 0),
                            stop=(dt_ == NDT - 1),
                        )
                    nc.scalar.activation(
                        out=out_tiles[et][:, ih * 512:(ih + 1) * 512],
                        in_=ps[:],
                        func=AF.Relu,
                        bias=bias_sb[:, et:et + 1],
                        scale=1.0,
                    )

        def s_phase(b, qTt, kTm, qm_sb):
            for ib in range(NIB):
                sp = spsum.tile([P, LK], F32, tag="S")
                for jh in range(NH):
                    for et in range(NET):
                        nc.tensor.matmul(
                            sp[:, jh * 512:(jh + 1) * 512],
                            lhsT=qTt[et][:, ib * P:(ib + 1) * P],
                            rhs=kTm[et][:, jh * 512:(jh + 1) * 512],
                            start=(et == 0),
                            stop=(et == NET - 1),
                        )
                ex = epool.tile([P, LK], BF16, tag="exp")
                rs = stpool.tile([P, 1], F32, tag="rowsum")
                nc.scalar.activation(
                    out=ex[:], in_=sp[:], func=AF.Exp, scale=SCALE,
                    accum_out=rs[:],
                )
                rc = stpool.tile([P, 1], F32, tag="recip")
                nc.vector.reciprocal(out=rc[:], in_=rs[:])
                rq = stpool.tile([P, 1], F32, tag="rq")
                nc.vector.tensor_tensor(
                    out=rq[:], in0=rc[:], in1=qm_sb[:, ib:ib + 1],
                    op=mybir.AluOpType.mult,
                )
                po = opool.tile([P, LK], F32, tag="po")
                nc.vector.tensor_scalar(
                    out=po[:], in0=ex[:], scalar1=rq[:], scalar2=None,
                    op0=mybir.AluOpType.mult,
                )
                nc.gpsimd.dma_start(
                    out=out[b, ib * P:(ib + 1) * P, :], in_=po[:]
                )

        cur = load_inputs(0)
        for b in range(BL):
            xk, mask_sb, xq, qm_sb = cur
            kraw = [actpool.tile([P, LK], BF16, tag=f"kraw{et}", name=f"kraw{et}_{b}")
                    for et in range(NET)]
            proj(xk, wk_sb, bk_sb, kraw)
            kTm = [actpool.tile([P, LK], BF16, tag=f"kTm{et}", name=f"kTm{et}_{b}")
                   for et in range(NET)]
            for et in range(NET):
                nc.gpsimd.tensor_add(kTm[et][:], kraw[et][:], mask_sb[:])
            qTt = [actpool.tile([P, LQ], BF16, tag=f"qT{et}", name=f"qT{et}_{b}")
                   for et in range(NET)]
            proj(xq, wq_sb, bq_sb, qTt)
            if b + 1 < BL:
                cur = load_inputs(b + 1)
            s_phase(b, qTt, kTm, qm_sb)


def _build():
    nc = bacc.Bacc(
        "TRN2",
        target_bir_lowering=False,
        debug=False,
        enable_asserts=False,
        num_devices=NCORES,
    )
    qT = nc.dram_tensor("qT", [BL, D, LQ], BF16, kind="ExternalInput").ap()
    kT = nc.dram_tensor("kT", [BL, D, LK], BF16, kind="ExternalInput").ap()
    Wq = nc.dram_tensor("Wq", [D, D], BF16, kind="ExternalInput").ap()
    Wk = nc.dram_tensor("Wk", [D, D], BF16, kind="ExternalInput").ap()
    bq = nc.dram_tensor("bq", [P, NET], F32, kind="ExternalInput").ap()
    bk = nc.dram_tensor("bk", [P, NET], F32, kind="ExternalInput").ap()
    maskb = nc.dram_tensor("maskb", [BL, P, LK], BF16, kind="ExternalInput").ap()
    qm = nc.dram_tensor("qm", [BL, P, NIB], F32, kind="ExternalInput").ap()
    out = nc.dram_tensor("out", [BL, LQ, LK], F32, kind="ExternalOutput").ap()

    with tile.TileContext(nc) as tc:
        _body(tc, qT, kT, Wq, Wk, bq, bk, maskb, qm, out)
    nc.compile()
    return nc


def _get_nc():
    if "nc" not in _CACHE:
        _CACHE["nc"] = _build()
    return _CACHE["nc"]


def _make_in_maps(query, key, query_mask, key_mask, Wq, bq, Wk, bk):
    bf = ml_dtypes.bfloat16
    query = np.asarray(query, dtype=np.float32)
    key = np.asarray(key, dtype=np.float32)
    query_mask = np.asarray(query_mask)
    key_mask = np.asarray(key_mask)
    Wq_b = np.asarray(Wq, dtype=np.float32).astype(bf)
    Wk_b = np.asarray(Wk, dtype=np.float32).astype(bf)
    # bias for feature e lives at partition e%128, column e//128
    bq_t = np.asarray(bq, dtype=np.float32).reshape(NET, P).T.copy()
    bk_t = np.asarray(bk, dtype=np.float32).reshape(NET, P).T.copy()

    in_maps = []
    for c in range(NCORES):
        sl = slice(c * BL, (c + 1) * BL)
        qTc = query[sl].transpose(0, 2, 1).astype(bf)
        kTc = key[sl].transpose(0, 2, 1).astype(bf)
        mrow = (MASKC * (1 - key_mask[sl])).astype(bf)            # [BL, LK]
        maskb = np.ascontiguousarray(
            np.broadcast_to(mrow[:, None, :], (BL, P, LK))
        )
        qmc = (
            query_mask[sl].astype(np.float32)
            .reshape(BL, NIB, P).transpose(0, 2, 1).copy()
        )
        in_maps.append({
            "qT": qTc, "kT": kTc, "Wq": Wq_b, "Wk": Wk_b,
            "bq": bq_t, "bk": bk_t, "maskb": maskb, "qm": qmc,
        })
    return in_maps


def run(query, key, query_mask, key_mask, Wq, bq, Wk, bk, **kwargs):
    """Run on hardware; returns (output, BassKernelResults)."""
    nc = _get_nc()
    in_maps = _make_in_maps(query, key, query_mask, key_mask, Wq, bq, Wk, bk)
    res = run_bass_kernel_spmd(nc, in_maps, core_ids=list(range(NCORES)), **kwargs)
    outs = [res.results[c]["out"] for c in range(NCORES)]
    full = np.concatenate(outs, axis=0).astype(np.float32, copy=False)
    return full, res


def kernel(query, key, query_mask, key_mask, Wq, bq, Wk, bk):
    full, _ = run(query, key, query_mask, key_mask, Wq, bq, Wk, bk)
    return full


# revision 5
# speedup vs baseline: 1.0142x; 1.0142x over previous
"""Masked attention-weight kernel (dense_transformer) for 8 TRN2 NeuronCores.

Computes, for inputs query/key [32,1024,512] f32, masks [32,1024] i32:
    q = relu(query @ Wq + bq); k = relu(key @ Wk + bk)
    w = softmax((q @ k^T)/sqrt(512) + key_mask_additive) * query_mask
Output: [32, 1024, 1024] f32.

Strategy: pure data-parallel over batch (4 batches/core, no collectives).
Host pre-transposes query/key to [B_local, D, L] and casts to bf16 so every
device matmul is transpose-free; compute is bf16 with f32 PSUM accumulation.
Softmax skips max-subtraction (logits bounded ~+-12; exp safe in f32). The
key mask is applied additively (-1e6) to the post-relu k-projection, which
makes masked logits ~-2e8 so exp underflows to exactly 0, and the ACT exp's
fused accum_out produces the masked row sum for free.
"""

import sys

sys.path.insert(0, "/opt/trn_rl_repo")

import numpy as np
import ml_dtypes
from contextlib import ExitStack

import concourse.bass as bass
import concourse.tile as tile
from concourse import bacc, mybir
from concourse.bass_utils import run_bass_kernel_spmd

P = 128
B, LQ, LK, D = 32, 1024, 1024, 512
NCORES = 8
BL = B // NCORES          # batches per core
NDT = D // P              # contraction tiles for projections
NET = D // P              # output-feature tiles (= S contraction tiles)
NIB = LQ // P             # 128-row blocks of S per batch
NH = LK // 512            # 512-col halves
SCALE = float(1.0 / np.sqrt(D))
MASKC = -1.0e6

F32 = mybir.dt.float32
BF16 = mybir.dt.bfloat16
AF = mybir.ActivationFunctionType

_CACHE = {}


def _body(tc, qT, kT, Wq, Wk, bq, bk, maskb, qm, out):
    nc = tc.nc
    with ExitStack() as ctx:
        consts = ctx.enter_context(tc.tile_pool(name="consts", bufs=1))
        wpool = ctx.enter_context(tc.tile_pool(name="w", bufs=1))
        inpool = ctx.enter_context(tc.tile_pool(name="inp", bufs=2))
        actpool = ctx.enter_context(tc.tile_pool(name="act", bufs=2))
        mpool = ctx.enter_context(tc.tile_pool(name="mask", bufs=2))
        epool = ctx.enter_context(tc.tile_pool(name="exp", bufs=3))
        opool = ctx.enter_context(tc.tile_pool(name="pout", bufs=3))
        stpool = ctx.enter_context(tc.tile_pool(name="stat", bufs=6))
        ppsum = ctx.enter_context(tc.tile_pool(name="ppsum", bufs=3, space="PSUM"))
        spsum = ctx.enter_context(tc.tile_pool(name="spsum", bufs=2, space="PSUM"))

        # Weights/biases issue from the scalar queue, inputs from sync —
        # separate SWDGE queues so the first k-proj deps (wk, xk0) land fast.
        bk_sb = consts.tile([P, NET], F32)
        nc.scalar.dma_start(out=bk_sb[:], in_=bk[:])
        wk_sb = []
        for dt_ in range(NDT):
            t = wpool.tile([P, D], BF16, tag=f"wk{dt_}")
            nc.scalar.dma_start(out=t[:], in_=Wk[dt_ * P:(dt_ + 1) * P, :])
            wk_sb.append(t)
        bq_sb = consts.tile([P, NET], F32)
        nc.scalar.dma_start(out=bq_sb[:], in_=bq[:])
        wq_sb = []
        for dt_ in range(NDT):
            t = wpool.tile([P, D], BF16, tag=f"wq{dt_}")
            nc.scalar.dma_start(out=t[:], in_=Wq[dt_ * P:(dt_ + 1) * P, :])
            wq_sb.append(t)

        def load_inputs(b):
            xk, xq = [], []
            for dt_ in range(NDT):
                t = inpool.tile([P, LK], BF16, tag=f"xk{dt_}")
                nc.sync.dma_start(out=t[:], in_=kT[b, dt_ * P:(dt_ + 1) * P, :])
                xk.append(t)
            mask_sb = mpool.tile([P, LK], BF16, tag="maskb")
            nc.scalar.dma_start(out=mask_sb[:], in_=maskb[b])
            for dt_ in range(NDT):
                t = inpool.tile([P, LQ], BF16, tag=f"xq{dt_}")
                nc.sync.dma_start(out=t[:], in_=qT[b, dt_ * P:(dt_ + 1) * P, :])
                xq.append(t)
            qm_sb = mpool.tile([P, NIB], F32, tag="qm")
            nc.scalar.dma_start(out=qm_sb[:], in_=qm[b])
            return xk, mask_sb, xq, qm_sb

        def proj(xin, w_sb, bias_sb, out_tiles):
            # out_tiles[et][:, ih*512:+512] = relu(W[:,et].T @ x[:, ih] + b)
            for ih in range(NH):
                for et in range(NET):
                    ps = ppsum.tile([P, 512], F32, tag="proj")
                    for dt_ in range(NDT):
                        nc.tensor.matmul(
                            ps[:],
                            lhsT=w_sb[dt_][:, et * P:(et + 1) * P],
                            rhs=xin[dt_][:, ih * 512:(ih + 1) * 512],
                            start=(dt_ == 0),
                            stop=(dt_ == NDT - 1),
                        )
                    nc.scalar.activation(
                        out=out_tiles[et][:, ih * 512:(ih + 1) * 512],
                        in_=ps[:],
                        func=AF.Relu,
                        bias=bias_sb[:, et:et + 1],
                        scale=1.0,
                    )

        def s_phase(b, qTt, kTm, qm_sb):
            for ib in range(NIB):
                sp = spsum.tile([P, LK], F32, tag="S")
                for jh in range(NH):
                    for et in range(NET):
                        nc.tensor.matmul(
                            sp[:, jh * 512:(jh + 1) * 512],
                            lhsT=qTt[et][:, ib * P:(ib + 1) * P],
                            rhs=kTm[et][:, jh * 512:(jh + 1) * 512],
                            start=(et == 0),
                            stop=(et == NET - 1),
                        )
                ex = epool.tile([P, LK], BF16, tag="exp")
                rs = stpool.tile([P, 1], F32, tag="rowsum")
                nc.scalar.activation(
                    out=ex[:], in_=sp[:], func=AF.Exp, scale=SCALE,
                    accum_out=rs[:],
                )
                rc = stpool.tile([P, 1], F32, tag="recip")
                nc.vector.reciprocal(out=rc[:], in_=rs[:])
                rq = stpool.tile([P, 1], F32, tag="rq")
                nc.vector.tensor_tensor(
                    out=rq[:], in0=rc[:], in1=qm_sb[:, ib:ib + 1],
                    op=mybir.AluOpType.mult,
                )
                po = opool.tile([P, LK], F32, tag="po")
                nc.vector.tensor_scalar(
                    out=po[:], in0=ex[:], scalar1=rq[:], scalar2=None,
                    op0=mybir.AluOpType.mult,
                )
                nc.gpsimd.dma_start(
                    out=out[b, ib * P:(ib + 1) * P, :], in_=po[:]
                )

        cur = load_inputs(0)
        for b in range(BL):
            xk, mask_sb, xq, qm_sb = cur
            kraw = [actpool.tile([P, LK], BF16, tag=f"kraw{et}", name=f"kraw{et}_{b}")
                    for et in range(NET)]
            proj(xk, wk_sb, bk_sb, kraw)
            kTm = [actpool.tile([P, LK], BF16, tag=f"kTm{et}", name=f"kTm{et}_{b}")
                   for et in range(NET)]
            for et in range(NET):
                # split mask-adds across gpsimd and vector so neither gates S
                eng = nc.gpsimd if et % 2 == 0 else nc.vector
                eng.tensor_add(kTm[et][:], kraw[et][:], mask_sb[:])
            qTt = [actpool.tile([P, LQ], BF16, tag=f"qT{et}", name=f"qT{et}_{b}")
                   for et in range(NET)]
            proj(xq, wq_sb, bq_sb, qTt)
            if b + 1 < BL:
                cur = load_inputs(b + 1)
            s_phase(b, qTt, kTm, qm_sb)


def _build():
    nc = bacc.Bacc(
        "TRN2",
        target_bir_lowering=False,
        debug=False,
        enable_asserts=False,
        num_devices=NCORES,
    )
    qT = nc.dram_tensor("qT", [BL, D, LQ], BF16, kind="ExternalInput").ap()
    kT = nc.dram_tensor("kT", [BL, D, LK], BF16, kind="ExternalInput").ap()
    Wq = nc.dram_tensor("Wq", [D, D], BF16, kind="ExternalInput").ap()
    Wk = nc.dram_tensor("Wk", [D, D], BF16, kind="ExternalInput").ap()
    bq = nc.dram_tensor("bq", [P, NET], F32, kind="ExternalInput").ap()
    bk = nc.dram_tensor("bk", [P, NET], F32, kind="ExternalInput").ap()
    maskb = nc.dram_tensor("maskb", [BL, P, LK], BF16, kind="ExternalInput").ap()
    qm = nc.dram_tensor("qm", [BL, P, NIB], F32, kind="ExternalInput").ap()
    out = nc.dram_tensor("out", [BL, LQ, LK], F32, kind="ExternalOutput").ap()

    with tile.TileContext(nc) as tc:
        _body(tc, qT, kT, Wq, Wk, bq, bk, maskb, qm, out)
    nc.compile()
    return nc


def _get_nc():
    if "nc" not in _CACHE:
        _CACHE["nc"] = _build()
    return _CACHE["nc"]


def _make_in_maps(query, key, query_mask, key_mask, Wq, bq, Wk, bk):
    bf = ml_dtypes.bfloat16
    query = np.asarray(query, dtype=np.float32)
    key = np.asarray(key, dtype=np.float32)
    query_mask = np.asarray(query_mask)
    key_mask = np.asarray(key_mask)
    Wq_b = np.asarray(Wq, dtype=np.float32).astype(bf)
    Wk_b = np.asarray(Wk, dtype=np.float32).astype(bf)
    # bias for feature e lives at partition e%128, column e//128
    bq_t = np.asarray(bq, dtype=np.float32).reshape(NET, P).T.copy()
    bk_t = np.asarray(bk, dtype=np.float32).reshape(NET, P).T.copy()

    in_maps = []
    for c in range(NCORES):
        sl = slice(c * BL, (c + 1) * BL)
        qTc = query[sl].transpose(0, 2, 1).astype(bf)
        kTc = key[sl].transpose(0, 2, 1).astype(bf)
        mrow = (MASKC * (1 - key_mask[sl])).astype(bf)            # [BL, LK]
        maskb = np.ascontiguousarray(
            np.broadcast_to(mrow[:, None, :], (BL, P, LK))
        )
        qmc = (
            query_mask[sl].astype(np.float32)
            .reshape(BL, NIB, P).transpose(0, 2, 1).copy()
        )
        in_maps.append({
            "qT": qTc, "kT": kTc, "Wq": Wq_b, "Wk": Wk_b,
            "bq": bq_t, "bk": bk_t, "maskb": maskb, "qm": qmc,
        })
    return in_maps


def run(query, key, query_mask, key_mask, Wq, bq, Wk, bk, **kwargs):
    """Run on hardware; returns (output, BassKernelResults)."""
    nc = _get_nc()
    in_maps = _make_in_maps(query, key, query_mask, key_mask, Wq, bq, Wk, bk)
    res = run_bass_kernel_spmd(nc, in_maps, core_ids=list(range(NCORES)), **kwargs)
    outs = [res.results[c]["out"] for c in range(NCORES)]
    full = np.concatenate(outs, axis=0).astype(np.float32, copy=False)
    return full, res


def kernel(query, key, query_mask, key_mask, Wq, bq, Wk, bk):
    full, _ = run(query, key, query_mask, key_mask, Wq, bq, Wk, bk)
    return full
